# revision 4
# baseline (speedup 1.0000x reference)
"""AttentionRNN Trainium2 kernel: MHA + 2-layer Elman RNN + FC head.

Sharding: data-parallel over batch (B=32 -> 4 per core x 8 cores), weights
replicated. Everything fp16 on the PE (1.6e-3 rel err vs fp32 reference),
fp32 PSUM accumulation, fp32 biases applied on ScalarE during PSUM eviction.

Layout strategy (per core, B=4, S=512, E=H=512, NH=8, HD=64):
  - x DMA-transposed to xT [E(part), tok]; QT/KT computed as [E, tok]
    (bias per-partition on ACT), V in natural [tok, E] layout augmented
    with a ones-column per head so the AV matmul also yields the softmax
    denominator row.
  - scoresT [k(part), q] per (b,h); exp on ACT (no max-subtraction: scores
    are O(+-6)); AV matmul gives ctxT_aug [65, q]; denominator reciprocal
    broadcast across 64 partitions via a tiny ones-outer-product matmul.
  - RNN: h kept [H(part), B(free)]; weights are the stationary matmul
    operand (fp16 -> fast weight load). Layer-1 input projection is batched
    per 64-step window (cuts sequential weight traffic from 3 to 2 matrices
    per step); only last-step h1 feeds the FC head.
"""

import os
import sys

try:
    import concourse  # noqa: F401
except ImportError:
    sys.path.insert(0, "/opt/trn_rl_repo")

import numpy as np
from contextlib import ExitStack

import concourse.bass as bass
import concourse.mybir as mybir
import concourse.tile as tile
from concourse import bacc
from concourse.bass import ds, ts
from concourse import bass_utils

N_CORES = 8
B, S, E, H, NH, HD = 32, 512, 512, 512, 8, 64
BC = B // N_CORES          # batch per core = 4
TOK = BC * S               # tokens per core = 2048
EC = E // 128              # 4 partition chunks
WIN = 64                   # RNN window length
NWIN = S // WIN

F16 = mybir.dt.float16
F32 = mybir.dt.float32
AF = mybir.ActivationFunctionType


def build_nc(bfc_val: float):
    nc = bacc.Bacc("TRN2", target_bir_lowering=False, debug=False)

    x_d = nc.dram_tensor("x", [TOK, E], F16, kind="ExternalInput")
    w_names = ["wq", "wk", "wv", "wo", "wih0", "wih1", "whh0", "whh1"]
    w_d = {n: nc.dram_tensor(n, [128, EC, E], F16, kind="ExternalInput") for n in w_names}
    bq_d = nc.dram_tensor("bq", [128, EC], F32, kind="ExternalInput")
    bk_d = nc.dram_tensor("bk", [128, EC], F32, kind="ExternalInput")
    bo_d = nc.dram_tensor("bo", [128, EC], F32, kind="ExternalInput")
    b0_d = nc.dram_tensor("b0", [128, EC], F32, kind="ExternalInput")
    b1_d = nc.dram_tensor("b1", [128, EC, BC], F32, kind="ExternalInput")
    wfc_d = nc.dram_tensor("wfc", [128, EC], F16, kind="ExternalInput")
    out_d = nc.dram_tensor("out", [BC, 1], F32, kind="ExternalOutput")

    with tile.TileContext(nc) as tc:
        with ExitStack() as ctx:
            consts = ctx.enter_context(tc.tile_pool(name="consts", bufs=1))
            w_sb = {}
            for n in w_names:
                w_sb[n] = consts.tile([128, EC, E], F16, tag=f"w_{n}", name=f"w_{n}")
                nc.sync.dma_start(w_sb[n][:], w_d[n][:])
            bq_sb = consts.tile([128, EC], F32, tag="bq")
            bk_sb = consts.tile([128, EC], F32, tag="bk")
            bo_sb = consts.tile([128, EC], F32, tag="bo")
            b0_sb = consts.tile([128, EC], F32, tag="b0")
            b1_sb = consts.tile([128, EC, BC], F32, tag="b1")
            wfc_sb = consts.tile([128, EC], F16, tag="wfc")
            for sb, d in [(bq_sb, bq_d), (bk_sb, bk_d), (bo_sb, bo_d),
                          (b0_sb, b0_d), (b1_sb, b1_d), (wfc_sb, wfc_d)]:
                nc.sync.dma_start(sb[:], d[:])
            ones_sb = consts.tile([1, 64], F32, tag="ones")
            nc.vector.memset(ones_sb[:], 1.0)
            zeros_sb = consts.tile([128, EC, BC], F16, tag="zeros")
            nc.vector.memset(zeros_sb[:], 0.0)
            # U0 = Wih0 @ atten_out.T + (bih0+bhh0), laid [128, chunk, b, s]
            u0_sb = consts.tile([128, EC, BC, S], F16, tag="u0")

            # ---------------- Phase A: attention + U0 precompute ----------
            with ExitStack() as actx, nc.named_scope("attn"):
                xt_p = actx.enter_context(tc.tile_pool(name="xt", bufs=2))
                qt_p = actx.enter_context(tc.tile_pool(name="qt", bufs=2))
                kt_p = actx.enter_context(tc.tile_pool(name="kt", bufs=2))
                va_p = actx.enter_context(tc.tile_pool(name="va", bufs=2))
                et_p = actx.enter_context(tc.tile_pool(name="et", bufs=2))
                cx_p = actx.enter_context(tc.tile_pool(name="cx", bufs=2))
                at_p = actx.enter_context(tc.tile_pool(name="at", bufs=2))
                rp_p = actx.enter_context(tc.tile_pool(name="rp", bufs=2))
                bs_p = actx.enter_context(tc.tile_pool(name="bs", bufs=2))
                pj_p = actx.enter_context(tc.tile_pool(name="pj", bufs=2, space="PSUM"))
                ps_p = actx.enter_context(tc.tile_pool(name="ps", bufs=2, space="PSUM"))
                pa_p = actx.enter_context(tc.tile_pool(name="pa", bufs=2, space="PSUM"))
                pb_p = actx.enter_context(tc.tile_pool(name="pb", bufs=2, space="PSUM"))

                for b in range(BC):
                    xT = xt_p.tile([128, EC, E], F16, tag="xt")
                    for m in range(EC):
                        nc.sync.dma_start_transpose(
                            xT[:, m, :], x_d[ds(b * S, S), ts(m, 128)]
                        )
                    QT = qt_p.tile([128, EC, S], F16, tag="qt")
                    KT = kt_p.tile([128, EC, S], F16, tag="kt")
                    for wname, bias_sb, dest in [("wq", bq_sb, QT), ("wk", bk_sb, KT)]:
                        for m in range(EC):
                            p = pj_p.tile([128, 512], F32, tag="pj")
                            for k in range(EC):
                                nc.tensor.matmul(
                                    p[:], w_sb[wname][:, k, ts(m, 128)], xT[:, k, :],
                                    start=(k == 0), stop=(k == EC - 1),
                                )
                            nc.scalar.activation(
                                dest[:, m, :], p[:], AF.Identity,
                                bias=bias_sb[:, m, None],
                            )
                    VA = va_p.tile([128, EC, NH, HD + 1], F16, tag="va")
                    for n in range(EC):
                        pv = pj_p.tile([128, NH, HD], F32, tag="pj")
                        for k in range(EC):
                            nc.tensor.matmul(
                                pv[:], xT[:, k, ts(n, 128)], w_sb["wv"][:, k, :],
                                start=(k == 0), stop=(k == EC - 1),
                            )
                        nc.vector.tensor_copy(out=VA[:, n, :, 0:HD], in_=pv[:])
                        nc.vector.memset(VA[:, n, :, HD], 1.0)

                    CX = cx_p.tile([128, EC, S], F16, tag="cx")
                    for h in range(NH):
                        po, chn = (h % 2) * 64, h // 2
                        ET = et_p.tile([128, EC, S], F16, tag="et")
                        for km in range(EC):
                            sp = ps_p.tile([128, 512], F32, tag="ps")
                            nc.tensor.matmul(
                                sp[:],
                                KT[po:po + 64, chn, ts(km, 128)],
                                QT[po:po + 64, chn, :],
                                start=True, stop=True,
                            )
                            nc.scalar.activation(ET[:, km, :], sp[:], AF.Exp)
                        av = pa_p.tile([128, 512], F32, tag="pa")
                        for km in range(EC):
                            nc.tensor.matmul(
                                av[:HD + 1, :], VA[:, km, h, :], ET[:, km, :],
                                start=(km == 0), stop=(km == EC - 1),
                            )
                        rp = rp_p.tile([1, 512], F32, tag="rp")
                        nc.vector.reciprocal(rp[:], av[HD:HD + 1, :])
                        pb = pb_p.tile([64, 512], F32, tag="pb")
                        nc.tensor.matmul(pb[:], ones_sb[:], rp[:], start=True, stop=True)
                        bs = bs_p.tile([64, 512], F32, tag="bs")
                        nc.vector.tensor_copy(out=bs[:], in_=pb[:])
                        nc.vector.tensor_mul(
                            out=CX[po:po + 64, chn, :], in0=av[:HD, :], in1=bs[:]
                        )
                    AT = at_p.tile([128, EC, S], F16, tag="at")
                    for m in range(EC):
                        p = pj_p.tile([128, 512], F32, tag="pj")
                        for k in range(EC):
                            nc.tensor.matmul(
                                p[:], w_sb["wo"][:, k, ts(m, 128)], CX[:, k, :],
                                start=(k == 0), stop=(k == EC - 1),
                            )
                        nc.scalar.activation(
                            AT[:, m, :], p[:], AF.Identity, bias=bo_sb[:, m, None]
                        )
                    for m in range(EC):
                        p = pj_p.tile([128, 512], F32, tag="pj")
                        for k in range(EC):
                            nc.tensor.matmul(
                                p[:], w_sb["wih0"][:, k, ts(m, 128)], AT[:, k, :],
                                start=(k == 0), stop=(k == EC - 1),
                            )
                        nc.scalar.activation(
                            u0_sb[:, m, b, :], p[:], AF.Identity, bias=b0_sb[:, m, None]
                        )

            # ---------------- Phase B: sequential RNN ---------------------
            with ExitStack() as rctx, nc.named_scope("rnn"):
                h0w_p = rctx.enter_context(tc.tile_pool(name="h0w", bufs=2))
                h1_p = rctx.enter_context(tc.tile_pool(name="h1", bufs=3))
                pre_p = rctx.enter_context(tc.tile_pool(name="pre", bufs=2))
                os_p = rctx.enter_context(tc.tile_pool(name="os", bufs=1))
                pl0_p = rctx.enter_context(tc.tile_pool(name="pl0", bufs=2, space="PSUM"))
                pl1_p = rctx.enter_context(tc.tile_pool(name="pl1", bufs=2, space="PSUM"))
                pw_p = rctx.enter_context(tc.tile_pool(name="pw", bufs=2, space="PSUM"))
                pf_p = rctx.enter_context(tc.tile_pool(name="pf", bufs=1, space="PSUM"))

                h0_src = (zeros_sb, None)   # (tile, t) ; t None -> [128, EC, BC] tile
                h1_prev = zeros_sb[:, :, :]
                for w in range(NWIN):
                    H0W = h0w_p.tile([128, EC, WIN, BC], F16, tag="h0w")
                    for t in range(WIN):
                        s = w * WIN + t
                        p0 = pl0_p.tile([128, EC, BC], F32, tag="pl0")
                        for m in range(EC):
                            for k in range(EC):
                                rhs = (h0_src[0][:, k, :] if h0_src[1] is None
                                       else h0_src[0][:, k, h0_src[1], :])
                                nc.tensor.matmul(
                                    p0[:, m, :], w_sb["whh0"][:, k, ts(m, 128)], rhs,
                                    start=(k == 0), stop=(k == EC - 1),
                                )
                        nc.vector.tensor_add(
                            out=p0[:], in0=p0[:], in1=u0_sb[:, :, :, s]
                        )
                        nc.scalar.activation(H0W[:, :, t, :], p0[:], AF.Tanh)
                        h0_src = (H0W, t)
                    PRE = pre_p.tile([128, EC, WIN, BC], F32, tag="pre")
                    for m in range(EC):
                        pwt = pw_p.tile([128, WIN, BC], F32, tag="pw")
                        for k in range(EC):
                            nc.tensor.matmul(
                                pwt[:], w_sb["wih1"][:, k, ts(m, 128)], H0W[:, k, :, :],
                                start=(k == 0), stop=(k == EC - 1),
                            )
                        nc.vector.tensor_add(
                            out=PRE[:, m], in0=pwt[:],
                            in1=b1_sb[:, m, None, :].to_broadcast((128, WIN, BC)),
                        )
                    for t in range(WIN):
                        p1 = pl1_p.tile([128, EC, BC], F32, tag="pl1")
                        for m in range(EC):
                            for k in range(EC):
                                nc.tensor.matmul(
                                    p1[:, m, :], w_sb["whh1"][:, k, ts(m, 128)],
                                    h1_prev[:, k, :],
                                    start=(k == 0), stop=(k == EC - 1),
                                )
                        nc.vector.tensor_add(
                            out=p1[:], in0=p1[:], in1=PRE[:, :, t, :]
                        )
                        h1_new = h1_p.tile([128, EC, BC], F16, tag="h1")
                        nc.scalar.activation(h1_new[:], p1[:], AF.Tanh)
                        h1_prev = h1_new[:, :, :]

                pf = pf_p.tile([BC, 1], F32, tag="pf")
                for k in range(EC):
                    nc.tensor.matmul(
                        pf[:], h1_prev[:, k, :], wfc_sb[:, k, None],
                        start=(k == 0), stop=(k == EC - 1),
                    )
                out_sb = os_p.tile([BC, 1], F32, tag="os")
                nc.scalar.activation(out_sb[:], pf[:], AF.Copy, bias=bfc_val)
                nc.sync.dma_start(out_d[:], out_sb[:])

    nc.compile()
    return nc


def _pack_w(wt: np.ndarray) -> np.ndarray:
    """[512,512] W.T (contraction-major) -> [128, EC, 512] fp16 chunk layout."""
    return np.ascontiguousarray(
        wt.reshape(EC, 128, E).transpose(1, 0, 2).astype(np.float16)
    )


def _pack_b(b: np.ndarray) -> np.ndarray:
    return np.ascontiguousarray(b.reshape(EC, 128).T.astype(np.float32))


def prepare_inputs(inputs):
    x = np.asarray(inputs["x"], dtype=np.float32)
    Wq, bq = np.asarray(inputs["Wq"]), np.asarray(inputs["bq"])
    Wk, bk = np.asarray(inputs["Wk"]), np.asarray(inputs["bk"])
    Wv, bv = np.asarray(inputs["Wv"]), np.asarray(inputs["bv"])
    Wo, bo = np.asarray(inputs["Wo"]), np.asarray(inputs["bo"])
    Wih, bih = np.asarray(inputs["Wih"]), np.asarray(inputs["bih"])
    Whh, bhh = np.asarray(inputs["Whh"]), np.asarray(inputs["bhh"])
    Wfc, bfc = np.asarray(inputs["Wfc"]), np.asarray(inputs["bfc"])

    shared = {
        "wq": _pack_w(Wq.T / np.sqrt(np.float32(HD))),
        "wk": _pack_w(Wk.T),
        "wv": _pack_w(Wv.T),
        "wo": _pack_w(Wo.T),
        "wih0": _pack_w(Wih[0].T),
        "wih1": _pack_w(Wih[1].T),
        "whh0": _pack_w(Whh[0].T),
        "whh1": _pack_w(Whh[1].T),
        "bq": _pack_b(bq / np.sqrt(np.float32(HD))),
        "bk": _pack_b(bk),
        "bo": _pack_b(bo + Wo @ bv),
        "b0": _pack_b(bih[0] + bhh[0]),
        "b1": np.ascontiguousarray(
            np.repeat(
                (bih[1] + bhh[1]).reshape(EC, 128).T[:, :, None], BC, axis=2
            ).astype(np.float32)
        ),
        "wfc": np.ascontiguousarray(
            Wfc[0].reshape(EC, 128).T.astype(np.float16)
        ),
    }
    x16 = x.astype(np.float16)
    in_maps = []
    for c in range(N_CORES):
        m = dict(shared)
        m["x"] = np.ascontiguousarray(
            x16[c * BC:(c + 1) * BC].reshape(TOK, E)
        )
        in_maps.append(m)
    return in_maps, float(bfc[0])


def run(inputs, trace=False):
    in_maps, bfc_val = prepare_inputs(inputs)
    nc = build_nc(bfc_val)
    if trace:
        _install_trace_shim()
        # the axon NTFF hook needs an initialized PJRT client: warm up with
        # an untraced execute first (also hides NEFF compile from the trace)
        bass_utils.run_bass_kernel_spmd(
            nc, in_maps, core_ids=list(range(N_CORES)), trace=False
        )
    res = bass_utils.run_bass_kernel_spmd(
        nc, in_maps, core_ids=list(range(N_CORES)), trace=trace,
        trace_cores=list(range(N_CORES)) if trace else None,
    )
    out = np.concatenate([res.results[c]["out"] for c in range(N_CORES)], axis=0)
    return out.astype(np.float32), res


def _install_trace_shim():
    """antenv.axon_hooks is missing in this image; recreate it so the axon
    NTFF profiling path in run_bass_kernel_spmd works."""
    import types
    mod = types.ModuleType("antenv.axon_hooks")
    holder = [None]
    mod.set_axon_ntff_profile_hook = lambda h: holder.__setitem__(0, h)
    mod.get_axon_ntff_profile_hook = lambda: holder[0]
    sys.modules["antenv.axon_hooks"] = mod
    try:
        import antenv
        antenv.axon_hooks = mod
    except ImportError:
        pass
    try:
        from trn_agent_boot.trn_boot import _ntff_profile_via_ctypes
        mod.set_axon_ntff_profile_hook(
            _ntff_profile_via_ctypes("/opt/axon/libaxon_pjrt.so")
        )
    except Exception:
        pass
    bass_utils.upload_artifacts = lambda d: "local://skipped"


def kernel(**inputs) -> np.ndarray:
    out, _ = run(inputs, trace=bool(os.environ.get("KERNEL_TRACE")))
    return out


# revision 6
# speedup vs baseline: 1.1964x; 1.1964x over previous
"""AttentionRNN Trainium2 kernel: MHA + 2-layer Elman RNN + FC head.

Sharding: data-parallel over batch (B=32 -> 4 per core x 8 cores), weights
replicated. Everything fp16 on the PE (1.6e-3 rel err vs fp32 reference),
fp32 PSUM accumulation, fp32 biases applied on ScalarE during PSUM eviction.

Layout strategy (per core, B=4, S=512, E=H=512, NH=8, HD=64):
  - x DMA-transposed to xT [E(part), tok]; QT/KT computed as [E, tok]
    (bias per-partition on ACT), V in natural [tok, E] layout augmented
    with a ones-column per head so the AV matmul also yields the softmax
    denominator row.
  - scoresT [k(part), q] per (b,h); exp on ACT (no max-subtraction: scores
    are O(+-6)); AV matmul gives ctxT_aug [65, q]; denominator reciprocal
    broadcast across 64 partitions via a tiny ones-outer-product matmul.
  - RNN: h kept [H(part), B(free)]; weights are the stationary matmul
    operand (fp16 -> fast weight load). Layer-1 input projection is batched
    per 64-step window (cuts sequential weight traffic from 3 to 2 matrices
    per step); only last-step h1 feeds the FC head.
"""

import os
import sys

try:
    import concourse  # noqa: F401
except ImportError:
    sys.path.insert(0, "/opt/trn_rl_repo")

import numpy as np
from contextlib import ExitStack

import concourse.bass as bass
import concourse.mybir as mybir
import concourse.tile as tile
from concourse import bacc
from concourse.bass import ds, ts
from concourse import bass_utils

N_CORES = 8
B, S, E, H, NH, HD = 32, 512, 512, 512, 8, 64
BC = B // N_CORES          # batch per core = 4
TOK = BC * S               # tokens per core = 2048
EC = E // 128              # 4 partition chunks
WIN = 64                   # RNN window length
NWIN = S // WIN

F16 = mybir.dt.float16
F32 = mybir.dt.float32
AF = mybir.ActivationFunctionType


def build_nc(bfc_val: float):
    nc = bacc.Bacc("TRN2", target_bir_lowering=False, debug=False)

    x_d = nc.dram_tensor("x", [TOK, E], F16, kind="ExternalInput")
    w_names = ["wq", "wk", "wv", "wo", "wih0", "wih1", "whh0", "whh1"]
    w_d = {n: nc.dram_tensor(n, [128, EC, E], F16, kind="ExternalInput") for n in w_names}
    bq_d = nc.dram_tensor("bq", [128, EC], F32, kind="ExternalInput")
    bk_d = nc.dram_tensor("bk", [128, EC], F32, kind="ExternalInput")
    bo_d = nc.dram_tensor("bo", [128, EC], F32, kind="ExternalInput")
    b0_d = nc.dram_tensor("b0", [128, EC], F32, kind="ExternalInput")
    b1_d = nc.dram_tensor("b1", [128, EC, BC], F32, kind="ExternalInput")
    wfc_d = nc.dram_tensor("wfc", [128, EC], F16, kind="ExternalInput")
    out_d = nc.dram_tensor("out", [BC, 1], F32, kind="ExternalOutput")

    with tile.TileContext(nc) as tc:
        with ExitStack() as ctx:
            consts = ctx.enter_context(tc.tile_pool(name="consts", bufs=1))
            w_sb = {}
            for n in w_names:
                w_sb[n] = consts.tile([128, EC, E], F16, tag=f"w_{n}", name=f"w_{n}")
                nc.sync.dma_start(w_sb[n][:], w_d[n][:])
            bq_sb = consts.tile([128, EC], F32, tag="bq")
            bk_sb = consts.tile([128, EC], F32, tag="bk")
            bo_sb = consts.tile([128, EC], F32, tag="bo")
            b0_sb = consts.tile([128, EC], F32, tag="b0")
            b1_sb = consts.tile([128, EC, BC], F32, tag="b1")
            wfc_sb = consts.tile([128, EC], F16, tag="wfc")
            for sb, d in [(bq_sb, bq_d), (bk_sb, bk_d), (bo_sb, bo_d),
                          (b0_sb, b0_d), (b1_sb, b1_d), (wfc_sb, wfc_d)]:
                nc.sync.dma_start(sb[:], d[:])
            ones_sb = consts.tile([1, 64], F32, tag="ones")
            nc.vector.memset(ones_sb[:], 1.0)
            zeros_sb = consts.tile([128, EC, BC], F16, tag="zeros")
            nc.vector.memset(zeros_sb[:], 0.0)
            # U0 = Wih0 @ atten_out.T + (bih0+bhh0), laid [128, chunk, b, s]
            u0_sb = consts.tile([128, EC, BC, S], F16, tag="u0")

            # ---------------- Phase A: attention + U0 precompute ----------
            with ExitStack() as actx, nc.named_scope("attn"):
                xt_p = actx.enter_context(tc.tile_pool(name="xt", bufs=2))
                qt_p = actx.enter_context(tc.tile_pool(name="qt", bufs=2))
                kt_p = actx.enter_context(tc.tile_pool(name="kt", bufs=2))
                va_p = actx.enter_context(tc.tile_pool(name="va", bufs=2))
                et_p = actx.enter_context(tc.tile_pool(name="et", bufs=2))
                cx_p = actx.enter_context(tc.tile_pool(name="cx", bufs=2))
                at_p = actx.enter_context(tc.tile_pool(name="at", bufs=2))
                rp_p = actx.enter_context(tc.tile_pool(name="rp", bufs=2))
                bs_p = actx.enter_context(tc.tile_pool(name="bs", bufs=2))
                pj_p = actx.enter_context(tc.tile_pool(name="pj", bufs=2, space="PSUM"))
                ps_p = actx.enter_context(tc.tile_pool(name="ps", bufs=2, space="PSUM"))
                pa_p = actx.enter_context(tc.tile_pool(name="pa", bufs=2, space="PSUM"))
                pb_p = actx.enter_context(tc.tile_pool(name="pb", bufs=2, space="PSUM"))

                for b in range(BC):
                    xT = xt_p.tile([128, EC, E], F16, tag="xt")
                    for m in range(EC):
                        nc.sync.dma_start_transpose(
                            xT[:, m, :], x_d[ds(b * S, S), ts(m, 128)]
                        )
                    QT = qt_p.tile([128, EC, S], F16, tag="qt")
                    KT = kt_p.tile([128, EC, S], F16, tag="kt")
                    for wname, bias_sb, dest in [("wq", bq_sb, QT), ("wk", bk_sb, KT)]:
                        for m in range(EC):
                            p = pj_p.tile([128, 512], F32, tag="pj")
                            for k in range(EC):
                                nc.tensor.matmul(
                                    p[:], w_sb[wname][:, k, ts(m, 128)], xT[:, k, :],
                                    start=(k == 0), stop=(k == EC - 1),
                                )
                            nc.scalar.activation(
                                dest[:, m, :], p[:], AF.Identity,
                                bias=bias_sb[:, m, None],
                            )
                    VA = va_p.tile([128, EC, NH, HD + 1], F16, tag="va")
                    for n in range(EC):
                        pv = pj_p.tile([128, NH, HD], F32, tag="pj")
                        for k in range(EC):
                            nc.tensor.matmul(
                                pv[:], xT[:, k, ts(n, 128)], w_sb["wv"][:, k, :],
                                start=(k == 0), stop=(k == EC - 1),
                            )
                        nc.vector.tensor_copy(out=VA[:, n, :, 0:HD], in_=pv[:])
                        nc.vector.memset(VA[:, n, :, HD], 1.0)

                    CX = cx_p.tile([128, EC, S], F16, tag="cx")
                    for h in range(NH):
                        po, chn = (h % 2) * 64, h // 2
                        ET = et_p.tile([128, EC, S], F16, tag="et")
                        for km in range(EC):
                            sp = ps_p.tile([128, 512], F32, tag="ps")
                            nc.tensor.matmul(
                                sp[:],
                                KT[po:po + 64, chn, ts(km, 128)],
                                QT[po:po + 64, chn, :],
                                start=True, stop=True,
                            )
                            nc.scalar.activation(ET[:, km, :], sp[:], AF.Exp)
                        av = pa_p.tile([128, 512], F32, tag="pa")
                        for km in range(EC):
                            nc.tensor.matmul(
                                av[:HD + 1, :], VA[:, km, h, :], ET[:, km, :],
                                start=(km == 0), stop=(km == EC - 1),
                            )
                        rp = rp_p.tile([1, 512], F32, tag="rp")
                        nc.vector.reciprocal(rp[:], av[HD:HD + 1, :])
                        pb = pb_p.tile([64, 512], F32, tag="pb")
                        nc.tensor.matmul(pb[:], ones_sb[:], rp[:], start=True, stop=True)
                        bs = bs_p.tile([64, 512], F32, tag="bs")
                        nc.vector.tensor_copy(out=bs[:], in_=pb[:])
                        nc.vector.tensor_mul(
                            out=CX[po:po + 64, chn, :], in0=av[:HD, :], in1=bs[:]
                        )
                    AT = at_p.tile([128, EC, S], F16, tag="at")
                    for m in range(EC):
                        p = pj_p.tile([128, 512], F32, tag="pj")
                        for k in range(EC):
                            nc.tensor.matmul(
                                p[:], w_sb["wo"][:, k, ts(m, 128)], CX[:, k, :],
                                start=(k == 0), stop=(k == EC - 1),
                            )
                        nc.scalar.activation(
                            AT[:, m, :], p[:], AF.Identity, bias=bo_sb[:, m, None]
                        )
                    for m in range(EC):
                        p = pj_p.tile([128, 512], F32, tag="pj")
                        for k in range(EC):
                            nc.tensor.matmul(
                                p[:], w_sb["wih0"][:, k, ts(m, 128)], AT[:, k, :],
                                start=(k == 0), stop=(k == EC - 1),
                            )
                        nc.scalar.activation(
                            u0_sb[:, m, b, :], p[:], AF.Identity, bias=b0_sb[:, m, None]
                        )

            # ---------------- Phase B: sequential RNN ---------------------
            with ExitStack() as rctx, nc.named_scope("rnn"):
                h0w_p = rctx.enter_context(tc.tile_pool(name="h0w", bufs=2))
                h1_p = rctx.enter_context(tc.tile_pool(name="h1", bufs=3))
                pre_p = rctx.enter_context(tc.tile_pool(name="pre", bufs=2))
                os_p = rctx.enter_context(tc.tile_pool(name="os", bufs=1))
                pl0_p = rctx.enter_context(tc.tile_pool(name="pl0", bufs=2, space="PSUM"))
                pl1_p = rctx.enter_context(tc.tile_pool(name="pl1", bufs=2, space="PSUM"))
                pw_p = rctx.enter_context(tc.tile_pool(name="pw", bufs=2, space="PSUM"))
                pf_p = rctx.enter_context(tc.tile_pool(name="pf", bufs=1, space="PSUM"))

                # Software-pipelined: L0 steps of window w interleave with
                # L1 steps of window w-1, so two independent dependency
                # chains keep the PE dense (HAM stays un-throttled).
                h0_src = (zeros_sb, None)   # (tile, t) ; t None -> [128, EC, BC] tile
                h1_prev = zeros_sb[:, :, :]
                H0W_done = None
                PRE = None
                for w in range(NWIN + 1):
                    if w > 0:
                        # batched Wih1 over the completed window w-1
                        PRE = pre_p.tile([128, EC, WIN, BC], F32, tag="pre")
                        for m in range(EC):
                            pwt = pw_p.tile([128, WIN, BC], F32, tag="pw")
                            for k in range(EC):
                                nc.tensor.matmul(
                                    pwt[:], w_sb["wih1"][:, k, ts(m, 128)],
                                    H0W_done[:, k, :, :],
                                    start=(k == 0), stop=(k == EC - 1),
                                )
                            nc.vector.tensor_add(
                                out=PRE[:, m], in0=pwt[:],
                                in1=b1_sb[:, m, None, :].to_broadcast((128, WIN, BC)),
                            )
                    H0W = (h0w_p.tile([128, EC, WIN, BC], F16, tag="h0w", name="h0w")
                           if w < NWIN else None)
                    for t in range(WIN):
                        if w < NWIN:
                            s = w * WIN + t
                            p0 = pl0_p.tile([128, EC, BC], F32, tag="pl0")
                            for m in range(EC):
                                for k in range(EC):
                                    rhs = (h0_src[0][:, k, :] if h0_src[1] is None
                                           else h0_src[0][:, k, h0_src[1], :])
                                    nc.tensor.matmul(
                                        p0[:, m, :], w_sb["whh0"][:, k, ts(m, 128)], rhs,
                                        start=(k == 0), stop=(k == EC - 1),
                                    )
                            nc.vector.tensor_add(
                                out=p0[:], in0=p0[:], in1=u0_sb[:, :, :, s]
                            )
                            nc.scalar.activation(H0W[:, :, t, :], p0[:], AF.Tanh)
                            h0_src = (H0W, t)
                        if w > 0:
                            p1 = pl1_p.tile([128, EC, BC], F32, tag="pl1")
                            for m in range(EC):
                                for k in range(EC):
                                    nc.tensor.matmul(
                                        p1[:, m, :], w_sb["whh1"][:, k, ts(m, 128)],
                                        h1_prev[:, k, :],
                                        start=(k == 0), stop=(k == EC - 1),
                                    )
                            nc.vector.tensor_add(
                                out=p1[:], in0=p1[:], in1=PRE[:, :, t, :]
                            )
                            h1_new = h1_p.tile([128, EC, BC], F16, tag="h1")
                            nc.scalar.activation(h1_new[:], p1[:], AF.Tanh)
                            h1_prev = h1_new[:, :, :]
                    if w < NWIN:
                        H0W_done = H0W

                pf = pf_p.tile([BC, 1], F32, tag="pf")
                for k in range(EC):
                    nc.tensor.matmul(
                        pf[:], h1_prev[:, k, :], wfc_sb[:, k, None],
                        start=(k == 0), stop=(k == EC - 1),
                    )
                out_sb = os_p.tile([BC, 1], F32, tag="os")
                nc.scalar.activation(out_sb[:], pf[:], AF.Copy, bias=bfc_val)
                nc.sync.dma_start(out_d[:], out_sb[:])

    nc.compile()
    return nc


def _pack_w(wt: np.ndarray) -> np.ndarray:
    """[512,512] W.T (contraction-major) -> [128, EC, 512] fp16 chunk layout."""
    return np.ascontiguousarray(
        wt.reshape(EC, 128, E).transpose(1, 0, 2).astype(np.float16)
    )


def _pack_b(b: np.ndarray) -> np.ndarray:
    return np.ascontiguousarray(b.reshape(EC, 128).T.astype(np.float32))


def prepare_inputs(inputs):
    x = np.asarray(inputs["x"], dtype=np.float32)
    Wq, bq = np.asarray(inputs["Wq"]), np.asarray(inputs["bq"])
    Wk, bk = np.asarray(inputs["Wk"]), np.asarray(inputs["bk"])
    Wv, bv = np.asarray(inputs["Wv"]), np.asarray(inputs["bv"])
    Wo, bo = np.asarray(inputs["Wo"]), np.asarray(inputs["bo"])
    Wih, bih = np.asarray(inputs["Wih"]), np.asarray(inputs["bih"])
    Whh, bhh = np.asarray(inputs["Whh"]), np.asarray(inputs["bhh"])
    Wfc, bfc = np.asarray(inputs["Wfc"]), np.asarray(inputs["bfc"])

    shared = {
        "wq": _pack_w(Wq.T / np.sqrt(np.float32(HD))),
        "wk": _pack_w(Wk.T),
        "wv": _pack_w(Wv.T),
        "wo": _pack_w(Wo.T),
        "wih0": _pack_w(Wih[0].T),
        "wih1": _pack_w(Wih[1].T),
        "whh0": _pack_w(Whh[0].T),
        "whh1": _pack_w(Whh[1].T),
        "bq": _pack_b(bq / np.sqrt(np.float32(HD))),
        "bk": _pack_b(bk),
        "bo": _pack_b(bo + Wo @ bv),
        "b0": _pack_b(bih[0] + bhh[0]),
        "b1": np.ascontiguousarray(
            np.repeat(
                (bih[1] + bhh[1]).reshape(EC, 128).T[:, :, None], BC, axis=2
            ).astype(np.float32)
        ),
        "wfc": np.ascontiguousarray(
            Wfc[0].reshape(EC, 128).T.astype(np.float16)
        ),
    }
    x16 = x.astype(np.float16)
    in_maps = []
    for c in range(N_CORES):
        m = dict(shared)
        m["x"] = np.ascontiguousarray(
            x16[c * BC:(c + 1) * BC].reshape(TOK, E)
        )
        in_maps.append(m)
    return in_maps, float(bfc[0])


def run(inputs, trace=False):
    in_maps, bfc_val = prepare_inputs(inputs)
    nc = build_nc(bfc_val)
    if trace:
        _install_trace_shim()
        # the axon NTFF hook needs an initialized PJRT client: warm up with
        # an untraced execute first (also hides NEFF compile from the trace)
        bass_utils.run_bass_kernel_spmd(
            nc, in_maps, core_ids=list(range(N_CORES)), trace=False
        )
    res = bass_utils.run_bass_kernel_spmd(
        nc, in_maps, core_ids=list(range(N_CORES)), trace=trace,
        trace_cores=list(range(N_CORES)) if trace else None,
    )
    out = np.concatenate([res.results[c]["out"] for c in range(N_CORES)], axis=0)
    return out.astype(np.float32), res


def _install_trace_shim():
    """antenv.axon_hooks is missing in this image; recreate it so the axon
    NTFF profiling path in run_bass_kernel_spmd works."""
    import types
    mod = types.ModuleType("antenv.axon_hooks")
    holder = [None]
    mod.set_axon_ntff_profile_hook = lambda h: holder.__setitem__(0, h)
    mod.get_axon_ntff_profile_hook = lambda: holder[0]
    sys.modules["antenv.axon_hooks"] = mod
    try:
        import antenv
        antenv.axon_hooks = mod
    except ImportError:
        pass
    try:
        from trn_agent_boot.trn_boot import _ntff_profile_via_ctypes
        mod.set_axon_ntff_profile_hook(
            _ntff_profile_via_ctypes("/opt/axon/libaxon_pjrt.so")
        )
    except Exception:
        pass
    bass_utils.upload_artifacts = lambda d: "local://skipped"


def kernel(**inputs) -> np.ndarray:
    out, _ = run(inputs, trace=bool(os.environ.get("KERNEL_TRACE")))
    return out
